# revision 12
# baseline (speedup 1.0000x reference)
"""MoE FFN (E=8 experts, top-2) — expert-parallel Bass/Tile kernel for 8 TRN2 cores.

Strategy (v2):
  - Host computes the (tiny) router: logits = x @ gate_w.T, top-2 per token,
    renormalized weights.  Token n is dispatched to cores e1(n), e2(n).
  - Device capacity C=1024 tokens per expert (= the perfectly balanced
    N*K/E share).  Overflow tokens beyond 1024 per expert (~1% of pairs)
    are computed exactly on the host during combine — load-balancing
    spillover, so every matmul on device has a full 512-token free dim.
  - All matmul operands fp16 (same 1 row/cycle PE rate as f32r on TRN2,
    half the DMA traffic and SBUF footprint; rel-err budget 2e-2 vs
    ~1e-3 realized).
  - mm1: hT[hc,c] = gelu(w1T-chunk @ xgT-chunk + b1), 32x2 chains of 8
    matmuls, PSUM free dim 512.
  - mm2: computes Y^T [D, C]: for each (dc, c), chain of 32 matmuls
    accumulating over hc with lhsT = w2 natural tiles.  Gate scaling and
    the top-2 combine (plus b2) happen on the host in fp32.
  - Engine assignment keeps queues independent: PE = matmuls,
    scalar(Act) = gelu + w2/b1 DMA ring, sync(SP) = xg/w1/output DMA
    ring, vector(DVE) = PSUM evictions.
"""

import re

import numpy as np

import bass_rust
import concourse.bass as bass
import concourse.mybir as mybir
import concourse.tile as tile
from concourse import bacc, bass_utils

P = 128
D_MODEL = 1024
D_HID = 4096
E = 8
TOP_K = 2
N_CORES = 8

DC = D_MODEL // P          # 8 d-chunks
HC = D_HID // P            # 32 h-chunks
C = 1024                   # device per-expert token capacity (rest -> host)
NCH = C // 512             # 2 token chunks of 512 (one PSUM bank each)

F32 = mybir.dt.float32
F16 = mybir.dt.float16
MM_DT = F16

W2_BUFS = 3                # w2 stream depth (tiles of [128, 4096] fp16)


_tail_patched = False


def _patch_light_tail():
    """Replace Tile's end-of-context machinery (multi-wait drain + two
    all-engine EVSEM barriers + semaphore range-clears, ~10us on HW) with
    single-wait drains on the sync engine covering every logical proc's final
    tick.  The NEFF is executed once per load in this flow, so semaphores
    need not be recycled."""
    global _tail_patched
    if _tail_patched:
        return
    _tail_patched = True

    def _drain_and_barrier(self, tick_clock, wait_clock):
        gc = tick_clock.global_clock
        ticks = eval(re.match(r"VectorClock\((.*)\)", repr(gc)).group(1))
        n = len(ticks)
        for i, v in enumerate(ticks):
            if v > 0:
                vc = bass_rust.VectorClock(
                    [v if j == i else 0 for j in range(n)])
                w = self.nc.sync.drain()
                wait_clock.add_sem_waits(
                    w.ins,
                    bass_rust.ScopedClock({None: vc}),
                    bass_rust.ScopedClock({}),
                )
        popped = self.nc._tile_sem_poison_stack.pop()
        assert popped is self._sem_poison
    tile.TileContext._drain_and_barrier = _drain_and_barrier


def build_nc():
    _patch_light_tail()
    nc = bacc.Bacc("TRN2", target_bir_lowering=False, debug=False,
                   num_devices=N_CORES)

    # Inputs, pre-tiled on host into consumption order (contiguous DMAs):
    #   xgt [NCH, 2, P, 2048]  xgt[c, q, p, r*512+t] = Xg[c*512+t, (4q+r)*128+p]
    #   w1t [HC, P, DC*P]      w1t[hc, p, dc*128+j] = w1[dc*128+p, hc*128+j]
    #   w2t [DC, P, HC*P]      w2t[dc, p, hc*128+j] = w2[hc*128+p, dc*128+j]
    #   b1t [P, HC]            b1t[p, hc] = b1[hc*128+p]
    # Output:
    #   ygt [D, C]             ygt[d, n] = Y[n, d]   (pre-gate, fp16)
    xgt = nc.dram_tensor("xgt", [NCH, 2, P, 4 * 512], MM_DT, kind="ExternalInput")
    w1t = nc.dram_tensor("w1t", [HC, P, DC * P], MM_DT, kind="ExternalInput")
    w2t = nc.dram_tensor("w2t", [DC, P, HC * P], MM_DT, kind="ExternalInput")
    b1t = nc.dram_tensor("b1t", [P, HC], F32, kind="ExternalInput")
    ygt = nc.dram_tensor("ygt", [D_MODEL, C], MM_DT, kind="ExternalOutput")

    with tile.TileContext(nc) as tc:
        with (
            tc.tile_pool(name="const", bufs=1) as const,
            tc.tile_pool(name="xg", bufs=1) as xg_pool,
            tc.tile_pool(name="w1", bufs=1) as w1_pool,
            tc.tile_pool(name="w2", bufs=W2_BUFS) as w2_pool,
            tc.tile_pool(name="ht", bufs=1) as ht_pool,
            tc.tile_pool(name="yo", bufs=4) as yo_pool,
            tc.tile_pool(name="ps1", bufs=4, space="PSUM") as ps1,
            tc.tile_pool(name="ps2", bufs=4, space="PSUM") as ps2,
        ):
            b1_sb = const.tile([P, HC], F32, name="b1sb")
            nc.scalar.dma_start(out=b1_sb[:], in_=b1t[:, :])

            # Head DMAs: only what pass A (token chunk c0) needs — w1#0
            # in two 128KB halves plus the 8 xg-c0 tiles — split across
            # the two HWDGE rings in consumption order.  The shared DMA
            # bus is ~360GB/s with ~650ns/DMA ring pacing and ~900ns
            # completion-sem latency, so keeping the head burst small is
            # what lets the first chains stream without starving.
            xg_sb = {}
            w1_sb = {}

            def w1_load(hc, eng):
                t = w1_pool.tile([P, DC * P], MM_DT, name=f"w1_{hc}")
                eng.dma_start(out=t[:], in_=w1t[hc, :, :])
                w1_sb[hc] = t

            def xg_load(dc, c, eng):
                t = xg_pool.tile([P, 512], MM_DT, name=f"xg{dc}_{c}")
                eng.dma_start(out=t[:], in_=xgt[c, dc // 4, :,
                                                (dc % 4) * 512:
                                                (dc % 4 + 1) * 512])
                xg_sb[(dc, c)] = t

            w1_sb[0] = w1_pool.tile([P, DC * P], MM_DT, name="w1_0")
            nc.sync.dma_start(out=w1_sb[0][:, :4 * P],
                              in_=w1t[0, :, :4 * P])
            nc.scalar.dma_start(out=w1_sb[0][:, 4 * P:],
                                in_=w1t[0, :, 4 * P:])
            for dc in range(DC):
                xg_load(dc, 0, nc.sync if dc % 2 == 0 else nc.scalar)
            w1_load(1, nc.sync)
            w1_load(2, nc.scalar)

            # ---- mm1: hT[hc, c] = gelu(w1.T @ xgT + b1) ----
            # Two passes over the 512-token chunks: all 32 c0 chains,
            # then all 32 c1 chains.  All w1 tiles stay resident (64KB/
            # partition) so pass B re-reads them from SBUF; xg-c1 and w2
            # stream in during pass A when the bus is otherwise idle.
            ht_sb = {}
            w2_sb = {}
            for c in range(NCH):
                for hc in range(HC):
                    if c == 0:
                        if hc + 3 < HC:
                            nhc = hc + 3
                            w1_load(nhc, nc.sync if nhc % 2 == 1
                                    else nc.scalar)
                        if 1 <= hc <= DC:
                            xg_load(hc - 1, 1,
                                    nc.sync if hc % 2 == 0 else nc.scalar)
                        if hc == 12:
                            # w2 prefetch once the input burst has drained
                            for k in range(W2_BUFS):
                                t = w2_pool.tile([P, HC * P], MM_DT,
                                                 name="w2sb")
                                nc.scalar.dma_start(out=t[:],
                                                    in_=w2t[k, :, :])
                                w2_sb[k] = t
                    ps = ps1.tile([P, 512], F32, name="ps1")
                    for dc in range(DC):
                        nc.tensor.matmul(
                            ps[:],
                            lhsT=w1_sb[hc][:, dc * P:(dc + 1) * P],
                            rhs=xg_sb[(dc, c)][:],
                            start=(dc == 0),
                            stop=(dc == DC - 1),
                        )
                    ht = ht_pool.tile([P, 512], MM_DT, name=f"ht{hc}_{c}")
                    nc.scalar.activation(
                        ht[:], ps[:],
                        mybir.ActivationFunctionType.Gelu,
                        bias=b1_sb[:, hc:hc + 1],
                    )
                    ht_sb[(hc, c)] = ht

            # ---- mm2: Y^T[dc, c] = sum_hc w2tile.T @ hT[hc, c] ----
            for dc in range(DC):
                if dc + W2_BUFS < DC:
                    ndc = dc + W2_BUFS
                    t = w2_pool.tile([P, HC * P], MM_DT, name="w2sb")
                    nc.scalar.dma_start(out=t[:], in_=w2t[ndc, :, :])
                    w2_sb[ndc] = t
                for c in range(NCH):
                    last = (dc == DC - 1 and c == NCH - 1)
                    # The very last chain is split into two half-width
                    # chains so the first half's evict+DMA (~2.2us of
                    # fixed DGE+sem latency) overlaps the second half's
                    # matmuls, shortening the kernel tail.
                    splits = ((0, 256), (256, 256)) if last else ((0, 512),)
                    for s0, sw in splits:
                        ps = ps2.tile([P, 512], F32, name="ps2")
                        for hc in range(HC):
                            nc.tensor.matmul(
                                ps[:, :sw],
                                lhsT=w2_sb[dc][:, hc * P:(hc + 1) * P],
                                rhs=ht_sb[(hc, c)][:, s0:s0 + sw],
                                start=(hc == 0),
                                stop=(hc == HC - 1),
                            )
                        yo = yo_pool.tile([P, 512], MM_DT, name="yo")
                        nc.vector.tensor_scalar_mul(
                            yo[:, :sw], ps[:, :sw], 1.0)
                        nc.sync.dma_start(
                            out=ygt[dc * P:(dc + 1) * P,
                                    c * 512 + s0:c * 512 + s0 + sw],
                            in_=yo[:, :sw],
                        )
    nc.compile()
    return nc


_NC_CACHE = None
TRACE = False
LAST_RESULTS = None


def _get_nc():
    global _NC_CACHE
    if _NC_CACHE is None:
        _NC_CACHE = build_nc()
    return _NC_CACHE


def _erf(x):
    try:
        from scipy.special import erf
        return erf(x)
    except ImportError:
        import math
        return np.frompyfunc(math.erf, 1, 1)(x).astype(np.float64)


def kernel(x, gate_w, w1, b1, w2, b2):
    x = np.asarray(x, dtype=np.float32)
    gate_w = np.asarray(gate_w, dtype=np.float32)
    w1 = np.asarray(w1, dtype=np.float32)
    b1 = np.asarray(b1, dtype=np.float32)
    w2 = np.asarray(w2, dtype=np.float32)
    b2 = np.asarray(b2, dtype=np.float32)

    B, T, D = x.shape
    N = B * T
    xf = x.reshape(N, D)

    # ---- router (host; 0.05% of model FLOPs — the sharding decision) ----
    logits = xf @ gate_w.T                           # [N, E]
    order = np.argsort(-logits, axis=1, kind="stable")
    i1, i2 = order[:, 0], order[:, 1]
    l1 = logits[np.arange(N), i1].astype(np.float64)
    l2 = logits[np.arange(N), i2].astype(np.float64)
    g1 = (1.0 / (1.0 + np.exp(l2 - l1))).astype(np.float32)
    g2 = (1.0 - g1).astype(np.float32)

    # ---- dispatch: gather per-expert tokens, pre-tile (fp16) ----
    in_maps = []
    idx_per_e = []
    gv_per_e = []
    for e in range(E):
        sel1 = np.nonzero(i1 == e)[0]
        sel2 = np.nonzero(i2 == e)[0]
        idx = np.concatenate([sel1, sel2])
        gv = np.concatenate([g1[sel1], g2[sel2]])
        idx_per_e.append(idx)
        gv_per_e.append(gv)
        dev = min(idx.shape[0], C)

        xg = np.zeros((C, D), np.float32)
        xg[:dev] = xf[idx[:dev]]
        # [c, q, p, r, t]: xgt[c, q, p, r*512+t] = Xg[c*512+t, (4q+r)*128+p]
        xgt = np.ascontiguousarray(
            xg.T.reshape(2, 4, P, NCH, 512).transpose(3, 0, 2, 1, 4).reshape(
                NCH, 2, P, 4 * 512)).astype(np.float16)
        w1t = np.ascontiguousarray(
            w1[e].reshape(DC, P, HC, P).transpose(2, 1, 0, 3).reshape(
                HC, P, DC * P)).astype(np.float16)
        w2t = np.ascontiguousarray(
            w2[e].reshape(HC, P, DC, P).transpose(2, 1, 0, 3).reshape(
                DC, P, HC * P)).astype(np.float16)
        b1t = np.ascontiguousarray(b1[e].reshape(HC, P).T)
        in_maps.append({"xgt": xgt, "w1t": w1t, "w2t": w2t, "b1t": b1t})

    nc = _get_nc()
    res = bass_utils.run_bass_kernel_spmd(
        nc, in_maps, core_ids=list(range(N_CORES)), trace=TRACE)
    global LAST_RESULTS
    LAST_RESULTS = res

    # ---- combine (host): gate scale + top-2 sum; overflow tokens beyond
    # device capacity get their exact fp32 FFN here (~1% of pairs) ----
    out = np.zeros((N, D), np.float32)
    for e in range(E):
        idx = idx_per_e[e]
        gv = gv_per_e[e]
        dev = min(idx.shape[0], C)
        y = res.results[e]["ygt"][:, :dev].astype(np.float32).T  # [dev, D]
        out[idx[:dev]] += gv[:dev, None] * y
        if idx.shape[0] > C:
            xs = xf[idx[C:]]                                     # [S, D]
            hs = xs @ w1[e] + b1[e].reshape(1, D_HID)
            hs = 0.5 * hs * (1.0 + _erf(hs / np.sqrt(2.0)))
            ys = (hs @ w2[e]).astype(np.float32)
            out[idx[C:]] += gv[C:, None] * ys

    if np.any(b2):
        gate_full = np.zeros((N, E), np.float32)
        gate_full[np.arange(N), i1] = g1
        gate_full[np.arange(N), i2] = g2
        out += gate_full @ b2.reshape(E, D)

    return out.reshape(B, T, D)


# revision 13
# speedup vs baseline: 1.0164x; 1.0164x over previous
"""MoE FFN (E=8 experts, top-2) — expert-parallel Bass/Tile kernel for 8 TRN2 cores.

Strategy (v2):
  - Host computes the (tiny) router: logits = x @ gate_w.T, top-2 per token,
    renormalized weights.  Token n is dispatched to cores e1(n), e2(n).
  - Device capacity C=1024 tokens per expert (= the perfectly balanced
    N*K/E share).  Overflow tokens beyond 1024 per expert (~1% of pairs)
    are computed exactly on the host during combine — load-balancing
    spillover, so every matmul on device has a full 512-token free dim.
  - All matmul operands fp16 (same 1 row/cycle PE rate as f32r on TRN2,
    half the DMA traffic and SBUF footprint; rel-err budget 2e-2 vs
    ~1e-3 realized).
  - mm1: hT[hc,c] = gelu(w1T-chunk @ xgT-chunk + b1), 32x2 chains of 8
    matmuls, PSUM free dim 512.
  - mm2: computes Y^T [D, C]: for each (dc, c), chain of 32 matmuls
    accumulating over hc with lhsT = w2 natural tiles.  Gate scaling and
    the top-2 combine (plus b2) happen on the host in fp32.
  - Engine assignment keeps queues independent: PE = matmuls,
    scalar(Act) = gelu + w2/b1 DMA ring, sync(SP) = xg/w1/output DMA
    ring, vector(DVE) = PSUM evictions.
"""

import re

import numpy as np

import bass_rust
import concourse.bass as bass
import concourse.mybir as mybir
import concourse.tile as tile
from concourse import bacc, bass_utils

P = 128
D_MODEL = 1024
D_HID = 4096
E = 8
TOP_K = 2
N_CORES = 8

DC = D_MODEL // P          # 8 d-chunks
HC = D_HID // P            # 32 h-chunks
C = 1024                   # device per-expert token capacity (rest -> host)
NCH = C // 512             # 2 token chunks of 512 (one PSUM bank each)

F32 = mybir.dt.float32
F16 = mybir.dt.float16
MM_DT = F16

W2_BUFS = 3                # w2 stream depth (tiles of [128, 4096] fp16)


_tail_patched = False


def _patch_light_tail():
    """Replace Tile's end-of-context machinery (multi-wait drain + two
    all-engine EVSEM barriers + semaphore range-clears, ~10us on HW) with
    single-wait drains on the sync engine covering every logical proc's final
    tick.  The NEFF is executed once per load in this flow, so semaphores
    need not be recycled."""
    global _tail_patched
    if _tail_patched:
        return
    _tail_patched = True

    def _drain_and_barrier(self, tick_clock, wait_clock):
        gc = tick_clock.global_clock
        ticks = eval(re.match(r"VectorClock\((.*)\)", repr(gc)).group(1))
        n = len(ticks)
        for i, v in enumerate(ticks):
            if v > 0:
                vc = bass_rust.VectorClock(
                    [v if j == i else 0 for j in range(n)])
                w = self.nc.sync.drain()
                wait_clock.add_sem_waits(
                    w.ins,
                    bass_rust.ScopedClock({None: vc}),
                    bass_rust.ScopedClock({}),
                )
        popped = self.nc._tile_sem_poison_stack.pop()
        assert popped is self._sem_poison
    tile.TileContext._drain_and_barrier = _drain_and_barrier


def build_nc():
    _patch_light_tail()
    nc = bacc.Bacc("TRN2", target_bir_lowering=False, debug=False,
                   num_devices=N_CORES)

    # Inputs, pre-tiled on host into consumption order (contiguous DMAs):
    #   xgt [NCH, 2, P, 2048]  xgt[c, q, p, r*512+t] = Xg[c*512+t, (4q+r)*128+p]
    #   w1t [HC, P, DC*P]      w1t[hc, p, dc*128+j] = w1[dc*128+p, hc*128+j]
    #   w2t [DC, P, HC*P]      w2t[dc, p, hc*128+j] = w2[hc*128+p, dc*128+j]
    #   b1t [P, HC]            b1t[p, hc] = b1[hc*128+p]
    # Output:
    #   ygt [D, C]             ygt[d, n] = Y[n, d]   (pre-gate, fp16)
    xgt = nc.dram_tensor("xgt", [NCH, 2, P, 4 * 512], MM_DT, kind="ExternalInput")
    w1t = nc.dram_tensor("w1t", [HC, P, DC * P], MM_DT, kind="ExternalInput")
    w2t = nc.dram_tensor("w2t", [DC, P, HC * P], MM_DT, kind="ExternalInput")
    b1t = nc.dram_tensor("b1t", [P, HC], F32, kind="ExternalInput")
    ygt = nc.dram_tensor("ygt", [D_MODEL, C], MM_DT, kind="ExternalOutput")

    with tile.TileContext(nc) as tc:
        with (
            tc.tile_pool(name="const", bufs=1) as const,
            tc.tile_pool(name="xg", bufs=1) as xg_pool,
            tc.tile_pool(name="w1", bufs=1) as w1_pool,
            tc.tile_pool(name="w2", bufs=W2_BUFS) as w2_pool,
            tc.tile_pool(name="ht", bufs=1) as ht_pool,
            tc.tile_pool(name="yo", bufs=4) as yo_pool,
            tc.tile_pool(name="ps1", bufs=4, space="PSUM") as ps1,
            tc.tile_pool(name="ps2", bufs=4, space="PSUM") as ps2,
        ):
            b1_sb = const.tile([P, HC], F32, name="b1sb")
            nc.scalar.dma_start(out=b1_sb[:], in_=b1t[:, :])

            # Head DMAs: only what pass A (token chunk c0) needs — w1#0
            # in two 128KB halves plus the 8 xg-c0 tiles — split across
            # the two HWDGE rings in consumption order.  The shared DMA
            # bus is ~360GB/s with ~650ns/DMA ring pacing and ~900ns
            # completion-sem latency, so keeping the head burst small is
            # what lets the first chains stream without starving.
            xg_sb = {}
            w1_sb = {}

            def w1_load(hc, eng):
                t = w1_pool.tile([P, DC * P], MM_DT, name=f"w1_{hc}")
                eng.dma_start(out=t[:], in_=w1t[hc, :, :])
                w1_sb[hc] = t

            def xg_load(dc, c, eng):
                t = xg_pool.tile([P, 512], MM_DT, name=f"xg{dc}_{c}")
                eng.dma_start(out=t[:], in_=xgt[c, dc // 4, :,
                                                (dc % 4) * 512:
                                                (dc % 4 + 1) * 512])
                xg_sb[(dc, c)] = t

            w1_sb[0] = w1_pool.tile([P, DC * P], MM_DT, name="w1_0")
            nc.sync.dma_start(out=w1_sb[0][:, :4 * P],
                              in_=w1t[0, :, :4 * P])
            nc.scalar.dma_start(out=w1_sb[0][:, 4 * P:],
                                in_=w1t[0, :, 4 * P:])
            for dc in range(DC):
                xg_load(dc, 0, nc.sync if dc % 2 == 0 else nc.scalar)
            w1_load(1, nc.sync)
            w1_load(2, nc.scalar)

            # ---- mm1: hT[hc, c] = gelu(w1.T @ xgT + b1) ----
            # Two passes over the 512-token chunks: all 32 c0 chains,
            # then all 32 c1 chains.  All w1 tiles stay resident (64KB/
            # partition) so pass B re-reads them from SBUF; xg-c1 and w2
            # stream in during pass A when the bus is otherwise idle.
            ht_sb = {}
            w2_sb = {}
            for c in range(NCH):
                for hc in range(HC):
                    if c == 0:
                        if hc + 3 < HC:
                            nhc = hc + 3
                            w1_load(nhc, nc.sync if nhc % 2 == 1
                                    else nc.scalar)
                        if 1 <= hc <= DC:
                            xg_load(hc - 1, 1,
                                    nc.sync if hc % 2 == 0 else nc.scalar)
                        if hc in (14, 19, 24):
                            # w2 prefetch, spread out so the 1MB transfers
                            # never back up the w1 stream on the same ring
                            k = {14: 0, 19: 1, 24: 2}[hc]
                            t = w2_pool.tile([P, HC * P], MM_DT,
                                             name="w2sb")
                            nc.scalar.dma_start(out=t[:], in_=w2t[k, :, :])
                            w2_sb[k] = t
                    ps = ps1.tile([P, 512], F32, name="ps1")
                    for dc in range(DC):
                        nc.tensor.matmul(
                            ps[:],
                            lhsT=w1_sb[hc][:, dc * P:(dc + 1) * P],
                            rhs=xg_sb[(dc, c)][:],
                            start=(dc == 0),
                            stop=(dc == DC - 1),
                        )
                    ht = ht_pool.tile([P, 512], MM_DT, name=f"ht{hc}_{c}")
                    nc.scalar.activation(
                        ht[:], ps[:],
                        mybir.ActivationFunctionType.Gelu,
                        bias=b1_sb[:, hc:hc + 1],
                    )
                    ht_sb[(hc, c)] = ht

            # ---- mm2: Y^T[dc, c] = sum_hc w2tile.T @ hT[hc, c] ----
            for dc in range(DC):
                if dc + W2_BUFS < DC:
                    ndc = dc + W2_BUFS
                    t = w2_pool.tile([P, HC * P], MM_DT, name="w2sb")
                    nc.scalar.dma_start(out=t[:], in_=w2t[ndc, :, :])
                    w2_sb[ndc] = t
                for c in range(NCH):
                    last = (dc == DC - 1 and c == NCH - 1)
                    # The very last chain is split into two half-width
                    # chains so the first half's evict+DMA (~2.2us of
                    # fixed DGE+sem latency) overlaps the second half's
                    # matmuls, shortening the kernel tail.
                    splits = ((0, 256), (256, 256)) if last else ((0, 512),)
                    for s0, sw in splits:
                        ps = ps2.tile([P, 512], F32, name="ps2")
                        for hc in range(HC):
                            nc.tensor.matmul(
                                ps[:, :sw],
                                lhsT=w2_sb[dc][:, hc * P:(hc + 1) * P],
                                rhs=ht_sb[(hc, c)][:, s0:s0 + sw],
                                start=(hc == 0),
                                stop=(hc == HC - 1),
                            )
                        yo = yo_pool.tile([P, 512], MM_DT, name="yo")
                        nc.vector.tensor_scalar_mul(
                            yo[:, :sw], ps[:, :sw], 1.0)
                        nc.sync.dma_start(
                            out=ygt[dc * P:(dc + 1) * P,
                                    c * 512 + s0:c * 512 + s0 + sw],
                            in_=yo[:, :sw],
                        )
    nc.compile()
    return nc


_NC_CACHE = None
TRACE = False
LAST_RESULTS = None


def _get_nc():
    global _NC_CACHE
    if _NC_CACHE is None:
        _NC_CACHE = build_nc()
    return _NC_CACHE


def _erf(x):
    try:
        from scipy.special import erf
        return erf(x)
    except ImportError:
        import math
        return np.frompyfunc(math.erf, 1, 1)(x).astype(np.float64)


def kernel(x, gate_w, w1, b1, w2, b2):
    x = np.asarray(x, dtype=np.float32)
    gate_w = np.asarray(gate_w, dtype=np.float32)
    w1 = np.asarray(w1, dtype=np.float32)
    b1 = np.asarray(b1, dtype=np.float32)
    w2 = np.asarray(w2, dtype=np.float32)
    b2 = np.asarray(b2, dtype=np.float32)

    B, T, D = x.shape
    N = B * T
    xf = x.reshape(N, D)

    # ---- router (host; 0.05% of model FLOPs — the sharding decision) ----
    logits = xf @ gate_w.T                           # [N, E]
    order = np.argsort(-logits, axis=1, kind="stable")
    i1, i2 = order[:, 0], order[:, 1]
    l1 = logits[np.arange(N), i1].astype(np.float64)
    l2 = logits[np.arange(N), i2].astype(np.float64)
    g1 = (1.0 / (1.0 + np.exp(l2 - l1))).astype(np.float32)
    g2 = (1.0 - g1).astype(np.float32)

    # ---- dispatch: gather per-expert tokens, pre-tile (fp16) ----
    in_maps = []
    idx_per_e = []
    gv_per_e = []
    for e in range(E):
        sel1 = np.nonzero(i1 == e)[0]
        sel2 = np.nonzero(i2 == e)[0]
        idx = np.concatenate([sel1, sel2])
        gv = np.concatenate([g1[sel1], g2[sel2]])
        idx_per_e.append(idx)
        gv_per_e.append(gv)
        dev = min(idx.shape[0], C)

        xg = np.zeros((C, D), np.float32)
        xg[:dev] = xf[idx[:dev]]
        # [c, q, p, r, t]: xgt[c, q, p, r*512+t] = Xg[c*512+t, (4q+r)*128+p]
        xgt = np.ascontiguousarray(
            xg.T.reshape(2, 4, P, NCH, 512).transpose(3, 0, 2, 1, 4).reshape(
                NCH, 2, P, 4 * 512)).astype(np.float16)
        w1t = np.ascontiguousarray(
            w1[e].reshape(DC, P, HC, P).transpose(2, 1, 0, 3).reshape(
                HC, P, DC * P)).astype(np.float16)
        w2t = np.ascontiguousarray(
            w2[e].reshape(HC, P, DC, P).transpose(2, 1, 0, 3).reshape(
                DC, P, HC * P)).astype(np.float16)
        b1t = np.ascontiguousarray(b1[e].reshape(HC, P).T)
        in_maps.append({"xgt": xgt, "w1t": w1t, "w2t": w2t, "b1t": b1t})

    nc = _get_nc()
    res = bass_utils.run_bass_kernel_spmd(
        nc, in_maps, core_ids=list(range(N_CORES)), trace=TRACE)
    global LAST_RESULTS
    LAST_RESULTS = res

    # ---- combine (host): gate scale + top-2 sum; overflow tokens beyond
    # device capacity get their exact fp32 FFN here (~1% of pairs) ----
    out = np.zeros((N, D), np.float32)
    for e in range(E):
        idx = idx_per_e[e]
        gv = gv_per_e[e]
        dev = min(idx.shape[0], C)
        y = res.results[e]["ygt"][:, :dev].astype(np.float32).T  # [dev, D]
        out[idx[:dev]] += gv[:dev, None] * y
        if idx.shape[0] > C:
            xs = xf[idx[C:]]                                     # [S, D]
            hs = xs @ w1[e] + b1[e].reshape(1, D_HID)
            hs = 0.5 * hs * (1.0 + _erf(hs / np.sqrt(2.0)))
            ys = (hs @ w2[e]).astype(np.float32)
            out[idx[C:]] += gv[C:, None] * ys

    if np.any(b2):
        gate_full = np.zeros((N, E), np.float32)
        gate_full[np.arange(N), i1] = g1
        gate_full[np.arange(N), i2] = g2
        out += gate_full @ b2.reshape(E, D)

    return out.reshape(B, T, D)


# revision 15
# speedup vs baseline: 1.0216x; 1.0051x over previous
"""MoE FFN (E=8 experts, top-2) — expert-parallel Bass/Tile kernel for 8 TRN2 cores.

Strategy (v2):
  - Host computes the (tiny) router: logits = x @ gate_w.T, top-2 per token,
    renormalized weights.  Token n is dispatched to cores e1(n), e2(n).
  - Device capacity C=1024 tokens per expert (= the perfectly balanced
    N*K/E share).  Overflow tokens beyond 1024 per expert (~1% of pairs)
    are computed exactly on the host during combine — load-balancing
    spillover, so every matmul on device has a full 512-token free dim.
  - All matmul operands fp16 (same 1 row/cycle PE rate as f32r on TRN2,
    half the DMA traffic and SBUF footprint; rel-err budget 2e-2 vs
    ~1e-3 realized).
  - mm1: hT[hc,c] = gelu(w1T-chunk @ xgT-chunk + b1), 32x2 chains of 8
    matmuls, PSUM free dim 512.
  - mm2: computes Y^T [D, C]: for each (dc, c), chain of 32 matmuls
    accumulating over hc with lhsT = w2 natural tiles.  Gate scaling and
    the top-2 combine (plus b2) happen on the host in fp32.
  - Engine assignment keeps queues independent: PE = matmuls,
    scalar(Act) = gelu + w2/b1 DMA ring, sync(SP) = xg/w1/output DMA
    ring, vector(DVE) = PSUM evictions.
"""

import re

import numpy as np

import bass_rust
import concourse.bass as bass
import concourse.mybir as mybir
import concourse.tile as tile
from concourse import bacc, bass_utils

P = 128
D_MODEL = 1024
D_HID = 4096
E = 8
TOP_K = 2
N_CORES = 8

DC = D_MODEL // P          # 8 d-chunks
HC = D_HID // P            # 32 h-chunks
C = 1024                   # device per-expert token capacity (rest -> host)
NCH = C // 512             # 2 token chunks of 512 (one PSUM bank each)

F32 = mybir.dt.float32
F16 = mybir.dt.float16
MM_DT = F16

W2_BUFS = 3                # w2 stream depth (tiles of [128, 4096] fp16)


_tail_patched = False


def _patch_light_tail():
    """Replace Tile's end-of-context machinery (multi-wait drain + two
    all-engine EVSEM barriers + semaphore range-clears, ~10us on HW) with
    single-wait drains on the sync engine covering every logical proc's final
    tick.  The NEFF is executed once per load in this flow, so semaphores
    need not be recycled."""
    global _tail_patched
    if _tail_patched:
        return
    _tail_patched = True

    def _drain_and_barrier(self, tick_clock, wait_clock):
        gc = tick_clock.global_clock
        ticks = eval(re.match(r"VectorClock\((.*)\)", repr(gc)).group(1))
        n = len(ticks)
        for i, v in enumerate(ticks):
            if v > 0:
                vc = bass_rust.VectorClock(
                    [v if j == i else 0 for j in range(n)])
                w = self.nc.sync.drain()
                wait_clock.add_sem_waits(
                    w.ins,
                    bass_rust.ScopedClock({None: vc}),
                    bass_rust.ScopedClock({}),
                )
        popped = self.nc._tile_sem_poison_stack.pop()
        assert popped is self._sem_poison
    tile.TileContext._drain_and_barrier = _drain_and_barrier


def build_nc():
    _patch_light_tail()
    nc = bacc.Bacc("TRN2", target_bir_lowering=False, debug=False,
                   num_devices=N_CORES)

    # Inputs, pre-tiled on host into consumption order (contiguous DMAs):
    #   xgt [NCH, 2, P, 2048]  xgt[c, q, p, r*512+t] = Xg[c*512+t, (4q+r)*128+p]
    #   w1t [HC, P, DC*P]      w1t[hc, p, dc*128+j] = w1[dc*128+p, hc*128+j]
    #   w2t [DC, P, HC*P]      w2t[dc, p, hc*128+j] = w2[hc*128+p, dc*128+j]
    #   b1t [P, HC]            b1t[p, hc] = b1[hc*128+p]
    # Output:
    #   ygt [D, C]             ygt[d, n] = Y[n, d]   (pre-gate, fp16)
    xgt = nc.dram_tensor("xgt", [NCH, 2, P, 4 * 512], MM_DT, kind="ExternalInput")
    w1t = nc.dram_tensor("w1t", [HC, P, DC * P], MM_DT, kind="ExternalInput")
    w2t = nc.dram_tensor("w2t", [DC, P, HC * P], MM_DT, kind="ExternalInput")
    b1t = nc.dram_tensor("b1t", [P, HC], F32, kind="ExternalInput")
    ygt = nc.dram_tensor("ygt", [D_MODEL, C], MM_DT, kind="ExternalOutput")

    with tile.TileContext(nc) as tc:
        with (
            tc.tile_pool(name="const", bufs=1) as const,
            tc.tile_pool(name="xg", bufs=1) as xg_pool,
            tc.tile_pool(name="w1", bufs=1) as w1_pool,
            tc.tile_pool(name="w2", bufs=W2_BUFS) as w2_pool,
            tc.tile_pool(name="ht", bufs=1) as ht_pool,
            tc.tile_pool(name="yo", bufs=4) as yo_pool,
            tc.tile_pool(name="ps1", bufs=4, space="PSUM") as ps1,
            tc.tile_pool(name="ps2", bufs=4, space="PSUM") as ps2,
        ):
            b1_sb = const.tile([P, HC], F32, name="b1sb")
            nc.scalar.dma_start(out=b1_sb[:], in_=b1t[:, :])

            # Head DMAs: only what pass A (token chunk c0) needs — w1#0
            # in two 128KB halves plus the 8 xg-c0 tiles — split across
            # the two HWDGE rings in consumption order.  The shared DMA
            # bus is ~360GB/s with ~650ns/DMA ring pacing and ~900ns
            # completion-sem latency, so keeping the head burst small is
            # what lets the first chains stream without starving.
            xg_sb = {}
            w1_sb = {}

            def w1_load(hc, eng):
                t = w1_pool.tile([P, DC * P], MM_DT, name=f"w1_{hc}")
                eng.dma_start(out=t[:], in_=w1t[hc, :, :])
                w1_sb[hc] = t

            def xg_load(c, q, eng):
                # one 512KB DMA per (chunk, dc-quartet): the ring paces
                # DMAs at ~0.65-1us each regardless of size, so fewer,
                # bigger transfers reach the PE sooner
                t = xg_pool.tile([P, 4 * 512], MM_DT, name=f"xg{c}_{q}")
                eng.dma_start(out=t[:], in_=xgt[c, q, :, :])
                xg_sb[(c, q)] = t

            w1_sb[0] = w1_pool.tile([P, DC * P], MM_DT, name="w1_0")
            nc.sync.dma_start(out=w1_sb[0][:, :4 * P],
                              in_=w1t[0, :, :4 * P])
            nc.scalar.dma_start(out=w1_sb[0][:, 4 * P:],
                                in_=w1t[0, :, 4 * P:])
            xg_load(0, 0, nc.sync)
            xg_load(0, 1, nc.scalar)
            w1_load(1, nc.sync)
            w1_load(2, nc.scalar)

            # ---- mm1: hT[hc, c] = gelu(w1.T @ xgT + b1) ----
            # Two passes over the 512-token chunks: all 32 c0 chains,
            # then all 32 c1 chains.  All w1 tiles stay resident (64KB/
            # partition) so pass B re-reads them from SBUF; xg-c1 and w2
            # stream in during pass A when the bus is otherwise idle.
            ht_sb = {}
            w2_sb = {}
            for c in range(NCH):
                for hc in range(HC):
                    if c == 0:
                        if hc + 3 < HC:
                            nhc = hc + 3
                            w1_load(nhc, nc.sync if nhc % 2 == 1
                                    else nc.scalar)
                        if hc == 4:
                            xg_load(1, 0, nc.sync)
                        if hc == 6:
                            xg_load(1, 1, nc.scalar)
                        if hc in (14, 19, 24):
                            # w2 prefetch, spread out so the 1MB transfers
                            # never back up the w1 stream on the same ring
                            k = {14: 0, 19: 1, 24: 2}[hc]
                            t = w2_pool.tile([P, HC * P], MM_DT,
                                             name="w2sb")
                            nc.scalar.dma_start(out=t[:], in_=w2t[k, :, :])
                            w2_sb[k] = t
                    ps = ps1.tile([P, 512], F32, name="ps1")
                    for dc in range(DC):
                        nc.tensor.matmul(
                            ps[:],
                            lhsT=w1_sb[hc][:, dc * P:(dc + 1) * P],
                            rhs=xg_sb[(c, dc // 4)][
                                :, (dc % 4) * 512:(dc % 4 + 1) * 512],
                            start=(dc == 0),
                            stop=(dc == DC - 1),
                        )
                    ht = ht_pool.tile([P, 512], MM_DT, name=f"ht{hc}_{c}")
                    nc.scalar.activation(
                        ht[:], ps[:],
                        mybir.ActivationFunctionType.Gelu,
                        bias=b1_sb[:, hc:hc + 1],
                    )
                    ht_sb[(hc, c)] = ht

            # ---- mm2: Y^T[dc, c] = sum_hc w2tile.T @ hT[hc, c] ----
            for dc in range(DC):
                if dc + W2_BUFS < DC:
                    ndc = dc + W2_BUFS
                    t = w2_pool.tile([P, HC * P], MM_DT, name="w2sb")
                    nc.scalar.dma_start(out=t[:], in_=w2t[ndc, :, :])
                    w2_sb[ndc] = t
                for c in range(NCH):
                    last = (dc == DC - 1 and c == NCH - 1)
                    # The very last chain is split into two half-width
                    # chains so the first half's evict+DMA (~2.2us of
                    # fixed DGE+sem latency) overlaps the second half's
                    # matmuls, shortening the kernel tail.
                    splits = ((0, 256), (256, 256)) if last else ((0, 512),)
                    for s0, sw in splits:
                        ps = ps2.tile([P, 512], F32, name="ps2")
                        for hc in range(HC):
                            nc.tensor.matmul(
                                ps[:, :sw],
                                lhsT=w2_sb[dc][:, hc * P:(hc + 1) * P],
                                rhs=ht_sb[(hc, c)][:, s0:s0 + sw],
                                start=(hc == 0),
                                stop=(hc == HC - 1),
                            )
                        yo = yo_pool.tile([P, 512], MM_DT, name="yo")
                        nc.vector.tensor_scalar_mul(
                            yo[:, :sw], ps[:, :sw], 1.0)
                        nc.sync.dma_start(
                            out=ygt[dc * P:(dc + 1) * P,
                                    c * 512 + s0:c * 512 + s0 + sw],
                            in_=yo[:, :sw],
                        )
    nc.compile()
    return nc


_NC_CACHE = None
TRACE = False
LAST_RESULTS = None


def _get_nc():
    global _NC_CACHE
    if _NC_CACHE is None:
        _NC_CACHE = build_nc()
    return _NC_CACHE


def _erf(x):
    try:
        from scipy.special import erf
        return erf(x)
    except ImportError:
        import math
        return np.frompyfunc(math.erf, 1, 1)(x).astype(np.float64)


def kernel(x, gate_w, w1, b1, w2, b2):
    x = np.asarray(x, dtype=np.float32)
    gate_w = np.asarray(gate_w, dtype=np.float32)
    w1 = np.asarray(w1, dtype=np.float32)
    b1 = np.asarray(b1, dtype=np.float32)
    w2 = np.asarray(w2, dtype=np.float32)
    b2 = np.asarray(b2, dtype=np.float32)

    B, T, D = x.shape
    N = B * T
    xf = x.reshape(N, D)

    # ---- router (host; 0.05% of model FLOPs — the sharding decision) ----
    logits = xf @ gate_w.T                           # [N, E]
    order = np.argsort(-logits, axis=1, kind="stable")
    i1, i2 = order[:, 0], order[:, 1]
    l1 = logits[np.arange(N), i1].astype(np.float64)
    l2 = logits[np.arange(N), i2].astype(np.float64)
    g1 = (1.0 / (1.0 + np.exp(l2 - l1))).astype(np.float32)
    g2 = (1.0 - g1).astype(np.float32)

    # ---- dispatch: gather per-expert tokens, pre-tile (fp16) ----
    in_maps = []
    idx_per_e = []
    gv_per_e = []
    for e in range(E):
        sel1 = np.nonzero(i1 == e)[0]
        sel2 = np.nonzero(i2 == e)[0]
        idx = np.concatenate([sel1, sel2])
        gv = np.concatenate([g1[sel1], g2[sel2]])
        idx_per_e.append(idx)
        gv_per_e.append(gv)
        dev = min(idx.shape[0], C)

        xg = np.zeros((C, D), np.float32)
        xg[:dev] = xf[idx[:dev]]
        # [c, q, p, r, t]: xgt[c, q, p, r*512+t] = Xg[c*512+t, (4q+r)*128+p]
        xgt = np.ascontiguousarray(
            xg.T.reshape(2, 4, P, NCH, 512).transpose(3, 0, 2, 1, 4).reshape(
                NCH, 2, P, 4 * 512)).astype(np.float16)
        w1t = np.ascontiguousarray(
            w1[e].reshape(DC, P, HC, P).transpose(2, 1, 0, 3).reshape(
                HC, P, DC * P)).astype(np.float16)
        w2t = np.ascontiguousarray(
            w2[e].reshape(HC, P, DC, P).transpose(2, 1, 0, 3).reshape(
                DC, P, HC * P)).astype(np.float16)
        b1t = np.ascontiguousarray(b1[e].reshape(HC, P).T)
        in_maps.append({"xgt": xgt, "w1t": w1t, "w2t": w2t, "b1t": b1t})

    nc = _get_nc()
    res = bass_utils.run_bass_kernel_spmd(
        nc, in_maps, core_ids=list(range(N_CORES)), trace=TRACE)
    global LAST_RESULTS
    LAST_RESULTS = res

    # ---- combine (host): gate scale + top-2 sum; overflow tokens beyond
    # device capacity get their exact fp32 FFN here (~1% of pairs) ----
    out = np.zeros((N, D), np.float32)
    for e in range(E):
        idx = idx_per_e[e]
        gv = gv_per_e[e]
        dev = min(idx.shape[0], C)
        y = res.results[e]["ygt"][:, :dev].astype(np.float32).T  # [dev, D]
        out[idx[:dev]] += gv[:dev, None] * y
        if idx.shape[0] > C:
            xs = xf[idx[C:]]                                     # [S, D]
            hs = xs @ w1[e] + b1[e].reshape(1, D_HID)
            hs = 0.5 * hs * (1.0 + _erf(hs / np.sqrt(2.0)))
            ys = (hs @ w2[e]).astype(np.float32)
            out[idx[C:]] += gv[C:, None] * ys

    if np.any(b2):
        gate_full = np.zeros((N, E), np.float32)
        gate_full[np.arange(N), i1] = g1
        gate_full[np.arange(N), i2] = g2
        out += gate_full @ b2.reshape(E, D)

    return out.reshape(B, T, D)
